# revision 1
# baseline (speedup 1.0000x reference)
"""FM pairwise-interaction layer on 8 Trainium2 NeuronCores.

out[b, p] = x[b, I1[p]] * x[b, I2[p]] * dot(w[I1[p]], w[I2[p]])   for all
P = 512*511/2 = 130816 strict upper-triangle pairs, batch 1024.

Strategy (data-parallel over batch, 128 rows per core):
  *  p-space is ordered by j1-blocks: block j1 covers columns
     [off(j1), off(j1)+n), n = 511-j1, with j2 = j1+1..511 contiguous.
  *  Host precomputes WP[k, p] = w[I1[p], k] * w[I2[p], k]  (weight-derived
     only, [4, P] fp32) and splits it into bf16 hi/lo.  Host also ships
     x.T in bf16 hi/lo, replicated per k, as the stationary operand.
  *  Per block, ONE K=12 bf16 matmul computes
        psum[b, c] = sum_k x[b, j1] * WP[k, off+c]  =  x[b, j1] * wdot[p]
     exactly-ish (hi*hi + hi*lo + lo*hi pairs, fp32 PSUM accumulate,
     ~1e-5 rel err).  Operands sit at 32-aligned partition groups.
  *  The second multiply (by the plain slice x[:, j1+1:512]) is split
     between the Vector engine (tensor_mul from PSUM) and, for every other
     block, a ScalarE PSUM->SBUF copy + GpSimd tensor_mul — balancing all
     engines below the HBM write roofline.
  *  Results land in flat-p staging chunks DMAd to DRAM as 2 MB transfers;
     the kernel is memory-bound on the 67 MB/core output write (~185 us).
"""

import numpy as np
import ml_dtypes

import concourse.bass as bass
import concourse.mybir as mybir
from concourse import bacc
from concourse.tile import TileContext
import concourse.bass_utils as bass_utils

NF = 512          # features
K = 4             # latent dim
B = 1024          # batch
NCORES = 8
BS = B // NCORES  # 128 batch rows per core
P = NF * (NF - 1) // 2  # 130816 pairs
CH = 2048         # staging chunk columns (fp32) -> 2 MB per DMA
STAGE_BUFS = 5
PSUM_BUFS = 4     # solo [128,512] 1-bank tiles
PAIR_BUFS = 2     # pair [128,1024] 2-bank tiles (4+2*2 = 8 banks total)
PAIR_MERGE = True # merge the two adjacent DVE blocks of each OFF_MOD=3 triple
# every OFF_MOD-th block takes the ACT-copy + GPSIMD-multiply path
# (relieves the DVE, which is otherwise the bottleneck alongside DMA)
OFFLOAD = True
OFF_MOD = 3
OFF_MIN_N = 32
REPS = 1          # replicate the main loop inside one NEFF (timing experiments)
# Device writes each [128, CH] chunk to a CONTIGUOUS DRAM range (chunk-major
# blob) instead of the 523KB-strided row-major layout: sequential HBM writes
# measure 9-18 us/pass faster. Host de-blocks after the gather.
CONTIG_OUT = True
TRACE = False
LAST_RESULT = {}
_last_in_maps = None

_bf16 = ml_dtypes.bfloat16


def _off(j1):
    return j1 * (NF - 1) - j1 * (j1 - 1) // 2


_GOFF = [_off(0), _off(128), _off(256), _off(384), P]
_GW = [_GOFF[g + 1] - _GOFF[g] for g in range(4)]  # 57280, 40896, 24512, 8128
_WPAD = 8


def _split_bf16(a):
    hi = a.astype(_bf16)
    lo = (a - hi.astype(np.float32)).astype(_bf16)
    return hi, lo


def ap2d(sliced, dims):
    """Copy of AP `sliced` with its free dims replaced by [step, count] pairs."""
    c = sliced.copy()
    v = c.ap
    part = [list(v[0])]
    while len(v) > 0:
        v.pop()
    for d in part + [list(x) for x in dims]:
        v.append(d)
    c.ap = v
    return c


def _build_nc():
    nc = bacc.Bacc("TRN2", target_bir_lowering=False, debug=False,
                   num_devices=NCORES)
    f32 = mybir.dt.float32
    bf16 = mybir.dt.bfloat16

    x_d = nc.dram_tensor("x", (BS, NF), f32, kind="ExternalInput").ap()
    xt_d = nc.dram_tensor("xt12", (4, 12, 128 * 128), bf16,
                          kind="ExternalInput").ap()
    wp_d = [nc.dram_tensor(f"wp{g}", (12, _GW[g] + _WPAD), bf16,
                           kind="ExternalInput").ap() for g in range(4)]
    out_d = nc.dram_tensor("out", (BS, P), f32, kind="ExternalOutput").ap()

    with TileContext(nc) as tc:
        with tc.tile_pool(name="sb", bufs=1) as sb, \
             tc.tile_pool(name="stg", bufs=STAGE_BUFS) as stg, \
             tc.tile_pool(name="sb2", bufs=3) as sb2, \
             tc.tile_pool(name="ps", bufs=PSUM_BUFS, space="PSUM") as ps, \
             tc.tile_pool(name="psp", bufs=PAIR_BUFS, space="PSUM") as psp:

            xs = sb.tile([128, NF + 8], f32, tag="xs")
            nc.vector.memset(xs[:, NF:NF + 8], 0.0)
            nc.sync.dma_start(out=xs[:, 0:NF], in_=x_d[:])

            xt = sb.tile([128, 128 * 128], bf16, tag="xt")
            wp = sb.tile([128, _GW[0] + _WPAD], bf16, tag="wp")
            for g in range(4):
                nc.sync.dma_start(out=xt[32 * g:32 * g + 12, :], in_=xt_d[g])
                nc.sync.dma_start(out=wp[32 * g:32 * g + 12, 0:_GW[g] + _WPAD],
                                  in_=wp_d[g][:])

            def lhs(j1):
                g = j1 // 128
                r = j1 - 128 * g
                return xt[32 * g:32 * g + 12, r * 128:(r + 1) * 128]

            def rhs(j1, n):
                g = j1 // 128
                lo = _off(j1) - _GOFF[g]
                return wp[32 * g:32 * g + 12, lo:lo + n]

            for _rep in range(REPS):
                main_pass(nc, stg, sb2, ps, psp, xs, lhs, rhs, out_d, f32)

    nc.compile()
    return nc


def main_pass(nc, stg, sb2, ps, psp, xs, lhs, rhs, out_d, f32):
            chunk_start = 0
            chunk_end = min(CH, P)
            stage = stg.tile([128, CH], f32, tag="stage")

            out_flat = out_d.rearrange("a b -> (a b)")

            def flush():
                nonlocal chunk_start, chunk_end, stage
                w = chunk_end - chunk_start
                if CONTIG_OUT:
                    dst = out_flat[chunk_start * 128:chunk_start * 128 + 128 * w]
                    dst = dst.rearrange("(p f) -> p f", p=128)
                    nc.sync.dma_start(out=dst, in_=stage[:, 0:w])
                else:
                    nc.sync.dma_start(out=out_d[:, chunk_start:chunk_end],
                                      in_=stage[:, 0:w])
                chunk_start = chunk_end
                chunk_end = min(chunk_start + CH, P)
                if chunk_start < P:
                    stage = stg.tile([128, CH], f32, tag="stage")

            j1 = 0
            while j1 < NF - 1:
                n = NF - 1 - j1
                o = _off(j1)
                g = j1 // 128
                # pair the two adjacent DVE blocks of each OFF_MOD triple
                pair_ok = (PAIR_MERGE and OFFLOAD and j1 % OFF_MOD == 0
                           and j1 + 1 < NF - 1
                           and (j1 + 1) % OFF_MOD != OFF_MOD - 1
                           and (j1 + 1) // 128 == g
                           and o >= chunk_start
                           and _off(j1 + 2) + 1 <= chunk_end)
                if pair_ok:
                    psum = psp.tile([128, 1024], f32, tag="psum_pair")
                    nc.tensor.matmul(psum[:, 0:n], lhs(j1), rhs(j1, n),
                                     start=True, stop=True,
                                     tile_position=(32 * g, 0))
                    # second block: n cols too (1 past its end; WP zero-padded)
                    nc.tensor.matmul(psum[:, 512:512 + n], lhs(j1 + 1),
                                     rhs(j1 + 1, n),
                                     start=True, stop=True,
                                     tile_position=(32 * g, 0))
                    lo = o - chunk_start
                    out_ap = ap2d(stage[:, lo:lo + 1], [[n, 2], [1, n]])
                    in0_ap = ap2d(psum[:, 0:1], [[512, 2], [1, n]])
                    in1_ap = ap2d(xs[:, j1 + 1:j1 + 2], [[1, 2], [1, n]])
                    nc.vector.tensor_mul(out=out_ap, in0=in0_ap, in1=in1_ap)
                    j1 += 2
                    if _off(j1) >= chunk_end:
                        flush()
                else:
                    psum = ps.tile([128, 512], f32, tag="psum")
                    nc.tensor.matmul(psum[:, 0:n], lhs(j1), rhs(j1, n),
                                     start=True, stop=True,
                                     tile_position=(32 * g, 0))
                    offl = (OFFLOAD and j1 % OFF_MOD == OFF_MOD - 1
                            and n >= OFF_MIN_N)
                    if offl:
                        tmp = sb2.tile([128, 512], f32, tag="tmp")
                        nc.scalar.copy(tmp[:, 0:n], psum[:, 0:n])
                        src = tmp
                    else:
                        src = psum
                    pos = o
                    while pos < o + n:
                        take = min(o + n, chunk_end) - pos
                        eng = nc.gpsimd if offl else nc.vector
                        eng.tensor_mul(
                            out=stage[:, pos - chunk_start:pos - chunk_start + take],
                            in0=src[:, pos - o:pos - o + take],
                            in1=xs[:, j1 + 1 + pos - o:j1 + 1 + pos - o + take])
                        pos += take
                        if pos == chunk_end:
                            flush()
                    j1 += 1
            if chunk_start < P:
                flush()


_NC_CACHE = None


def kernel(x, weight):
    global _NC_CACHE, LAST_RESULT
    x = np.ascontiguousarray(x, dtype=np.float32)
    weight = np.ascontiguousarray(weight, dtype=np.float32)
    assert x.shape == (B, NF) and weight.shape == (NF, K)

    # ---- host-side weight-derived constants
    i1, i2 = np.triu_indices(NF, k=1)
    wp_full = (weight[i1] * weight[i2]).T.astype(np.float32)  # [K, P]
    wph, wpl = _split_bf16(wp_full)
    wp_in = {}
    for g in range(4):
        arr = np.zeros((12, _GW[g] + _WPAD), dtype=_bf16)
        sl = slice(_GOFF[g], _GOFF[g + 1])
        for k in range(K):
            arr[3 * k + 0, 0:_GW[g]] = wph[k, sl]
            arr[3 * k + 1, 0:_GW[g]] = wpl[k, sl]
            arr[3 * k + 2, 0:_GW[g]] = wph[k, sl]
        wp_in[f"wp{g}"] = arr

    # ---- per-core inputs
    in_maps = []
    for c in range(NCORES):
        xc = x[c * BS:(c + 1) * BS]           # [128, 512]
        xct = np.ascontiguousarray(xc.T)      # [512, 128]
        xh, xl = _split_bf16(xct)
        xt12 = np.empty((4, 12, 128 * 128), dtype=_bf16)
        for g in range(4):
            fh = xh[128 * g:128 * (g + 1)].reshape(-1)
            fl = xl[128 * g:128 * (g + 1)].reshape(-1)
            for k in range(K):
                xt12[g, 3 * k + 0] = fh
                xt12[g, 3 * k + 1] = fh
                xt12[g, 3 * k + 2] = fl
        m = {"x": xc, "xt12": xt12}
        m.update(wp_in)
        in_maps.append(m)

    global _last_in_maps
    _last_in_maps = in_maps
    if _NC_CACHE is None:
        _NC_CACHE = _build_nc()
    nc = _NC_CACHE

    res = bass_utils.run_bass_kernel_spmd(nc, in_maps,
                                          core_ids=list(range(NCORES)),
                                          trace=TRACE)
    LAST_RESULT = {"exec_time_ns": res.exec_time_ns,
                   "trace": res.instructions_and_trace}
    if CONTIG_OUT:
        cores = []
        for r in res.results:
            blob = r["out"].reshape(-1)
            oc = np.empty((BS, P), np.float32)
            cs = 0
            while cs < P:
                w = min(CH, P - cs)
                oc[:, cs:cs + w] = blob[128 * cs:128 * (cs + w)].reshape(BS, w)
                cs += w
            cores.append(oc)
        out = np.concatenate(cores, axis=0)
    else:
        out = np.concatenate([r["out"] for r in res.results], axis=0)
    return out



# revision 6
# speedup vs baseline: 1.8900x; 1.8900x over previous
"""FM pairwise-interaction layer on 8 Trainium2 NeuronCores — bf16-out design.

out[b, p] = x[b, I1[p]] * x[b, I2[p]] * wdot[p],  wdot[p] = <w[I1p], w[I2p]>,
P = 512*511/2 = 130816 strict upper-triangle pairs, batch 1024.

Strategy (data-parallel over batch, 128 rows per core):
  *  wdot is computed on the host (weight-only, [P] fp32) and shipped as 4
     bf16 rows (hi/hi/lo/lo); x ships as x^T bf16 hi/lo stationaries.  Per
     j1-block one K=4 matmul makes psum[b, c] = x[b, j1] * wdot[off+c]
     (fp32 PSUM, ~1e-4 exact).
  *  Blocks are processed in GROUPS of 4 consecutive j1 (one PSUM bank
     each, padded to a common even width n_pad) so one evacuation
     instruction covers all 4 via a 2D access pattern: amortizes the
     120-220-cycle per-instruction engine overheads.
  *  Evacuation (the fused second multiply x[b, j2] + bf16 downcast) is
     split between two paths, balanced by a cost model:
       A: DVE tensor_mul straight from PSUM (fp32 1x mode)
       B: ACT copies psum -> bf16 SBUF, then DVE bf16 tensor_mul in 2x_1P
          packed mode (even/odd column parity handled via two shifted
          bf16 copies of x so every AP run start is 4B-aligned)
  *  Stage chunks are bf16 -> DRAM as a contiguous chunk-major blob; the
     host de-pads, de-blocks, and upcasts bf16->fp32 with a bit shift.
"""

import numpy as np
import ml_dtypes

import concourse.bass as bass
import concourse.mybir as mybir
from concourse import bacc
from concourse.tile import TileContext
import concourse.bass_utils as bass_utils

NF = 512          # features
K = 4             # latent dim
B = 1024          # batch
NCORES = 8
BS = B // NCORES  # 128 batch rows per core
P = NF * (NF - 1) // 2  # 130816 pairs

MODE = "full"     # full | dma | dve | act | pe   (bench modes)
CH = 4096         # stage chunk columns (bf16) -> 1 MB per DMA flush
STAGE_BUFS = 4
TMP_BUFS = 3
PSUM_BUFS = 2     # [128, 2048] 4-bank tiles
FLUSH_EVERY = 1   # benches set >1 to subsample DMA
# path-B fraction of columns (ACT+DVE2x); rest path A (DVE direct).
# Chosen per measured rates; see _assign_paths.
RATE_DVE1 = 0.96  # G cols/s payload, path A TT fp32-from-psum
RATE_DVE2 = 1.92  # G cols/s payload, path B TT bf16 2x
RATE_ACT = 1.2    # G cols/s payload, ACT psum->sbuf copy
OVH_DVE1 = 120.0  # cycles @0.96
OVH_DVE2 = 58.0
OVH_ACT = 172.0   # cycles @1.2
REPS = 1
TRACE = False
LAST_RESULT = {}
_last_in_maps = None

_bf16 = ml_dtypes.bfloat16

_XPAD = 8         # xs tiles padded to NF+_XPAD cols (OOB reads of pad cols)
_WPAD = 8


def _off(j1):
    return j1 * (NF - 1) - j1 * (j1 - 1) // 2


_GOFF = [_off(0), _off(128), _off(256), _off(384), P]
_GW = [_GOFF[g + 1] - _GOFF[g] for g in range(4)]  # 57280, 40896, 24512, 8128


def _build_groups():
    """Groups of <=4 consecutive j1 blocks, padded to even n_pad."""
    groups = []
    pp = 0
    j1 = 0
    while j1 < NF - 1:
        M = min(4, NF - 1 - j1)
        n0 = NF - 1 - j1
        n_pad = n0 + (n0 & 1)
        ns = [NF - 1 - (j1 + k) for k in range(M)]
        groups.append(dict(j1=j1, M=M, n_pad=n_pad, ns=ns, pp=pp))
        pp += M * n_pad
        j1 += M
    return groups, pp


_GROUPS, PPAD = _build_groups()


def _assign_paths():
    """Greedy per-group path choice minimizing projected engine makespan."""
    t_dve = 0.0
    t_act = 0.0
    out = []
    for g in _GROUPS:
        fd = g["M"] * g["n_pad"]
        # path A cost on DVE (ns)
        a_cost = (OVH_DVE1 + fd) / RATE_DVE1
        # path B: ACT copy + two DVE 2x TTs
        b_act = (OVH_ACT + fd) / RATE_ACT
        b_dve = 2 * OVH_DVE2 / 0.96 + fd / RATE_DVE2
        if max(t_dve + a_cost, t_act) <= max(t_dve + b_dve, t_act + b_act):
            out.append("A")
            t_dve += a_cost
        else:
            out.append("B")
            t_dve += b_dve
            t_act += b_act
    return out


def _split_bf16(a):
    hi = a.astype(_bf16)
    lo = (a - hi.astype(np.float32)).astype(_bf16)
    return hi, lo


def ap2d(sliced, dims):
    """Copy of AP `sliced` with its free dims replaced by [step, count]."""
    c = sliced.copy()
    v = c.ap
    part = [list(v[0])]
    while len(v) > 0:
        v.pop()
    for d in part + [list(x) for x in dims]:
        v.append(d)
    c.ap = v
    return c


def _build_nc():
    nc = bacc.Bacc("TRN2", target_bir_lowering=False, debug=False,
                   num_devices=NCORES)
    f32 = mybir.dt.float32
    bf16 = mybir.dt.bfloat16

    xt_d = nc.dram_tensor("xt4", (4, 4, 128 * 128), bf16,
                          kind="ExternalInput").ap()
    wp_d = [nc.dram_tensor(f"wp{g}", (4, _GW[g] + _WPAD), bf16,
                           kind="ExternalInput").ap() for g in range(4)]
    xse_d = nc.dram_tensor("xse", (BS, NF + _XPAD), bf16,
                           kind="ExternalInput").ap()
    xso_d = nc.dram_tensor("xso", (BS, NF + _XPAD), bf16,
                           kind="ExternalInput").ap()
    out_d = nc.dram_tensor("out", (BS, PPAD), bf16, kind="ExternalOutput").ap()

    paths = _assign_paths()

    with TileContext(nc) as tc:
        with tc.tile_pool(name="sb", bufs=1) as sb, \
             tc.tile_pool(name="stg", bufs=STAGE_BUFS) as stg, \
             tc.tile_pool(name="tmp", bufs=TMP_BUFS) as tmpp, \
             tc.tile_pool(name="ps", bufs=PSUM_BUFS, space="PSUM") as ps:

            xse = sb.tile([128, NF + _XPAD], bf16, tag="xse")
            xso = sb.tile([128, NF + _XPAD], bf16, tag="xso")
            nc.sync.dma_start(out=xse[:], in_=xse_d[:])
            nc.sync.dma_start(out=xso[:], in_=xso_d[:])

            xt = sb.tile([128, 128 * 128], bf16, tag="xt")
            wp = sb.tile([128, _GW[0] + _WPAD], bf16, tag="wp")
            for g in range(4):
                nc.sync.dma_start(out=xt[32 * g:32 * g + 4, :], in_=xt_d[g])
                nc.sync.dma_start(out=wp[32 * g:32 * g + 4, 0:_GW[g] + _WPAD],
                                  in_=wp_d[g][:])

            def lhs(j1):
                g = j1 // 128
                r = j1 - 128 * g
                return xt[32 * g:32 * g + 4, r * 128:(r + 1) * 128]

            def rhs(j1, n):
                g = j1 // 128
                lo = _off(j1) - _GOFF[g]
                return wp[32 * g:32 * g + 4, lo:lo + n]

            for _rep in range(REPS):
                main_pass(nc, stg, tmpp, ps, xse, xso, lhs, rhs, out_d,
                          paths, f32, bf16)

    nc.compile()
    return nc


def main_pass(nc, stg, tmpp, ps, xse, xso, lhs, rhs, out_d, paths, f32, bf16):
    out_flat = out_d.rearrange("a b -> (a b)")

    if MODE == "dma":
        stage = stg.tile([128, CH], bf16, tag="stage")
        nc.vector.memset(stage[:], 0.0)
        pos = 0
        while pos < PPAD:
            w = min(CH, PPAD - pos)
            dst = out_flat[pos * 128:pos * 128 + 128 * w]
            dst = dst.rearrange("(p f) -> p f", p=128)
            nc.sync.dma_start(out=dst, in_=stage[:, 0:w])
            pos += w
        return

    cur = 0          # used cols in current stage chunk
    chunk_base = 0   # padded-p offset of current chunk start
    nflush = 0
    if MODE != "pe":
        stage = stg.tile([128, CH], bf16, tag="stage")
    else:
        stage = None

    def flush():
        nonlocal cur, chunk_base, stage, nflush
        if cur == 0 or MODE == "pe":
            cur = 0
            return
        if nflush % FLUSH_EVERY == 0:
            dst = out_flat[chunk_base * 128:chunk_base * 128 + 128 * cur]
            dst = dst.rearrange("(p f) -> p f", p=128)
            nc.sync.dma_start(out=dst, in_=stage[:, 0:cur])
        nflush += 1
        chunk_base += cur
        cur = 0
        if chunk_base < PPAD:
            stage = stg.tile([128, CH], bf16, tag="stage")

    for gi, g in enumerate(_GROUPS):
        j1, M, n_pad, ns = g["j1"], g["M"], g["n_pad"], g["ns"]
        width = M * n_pad
        if cur + width > CH:
            flush()

        psum = ps.tile([128, 2048], f32, tag="psum")
        if MODE in ("full", "pe"):
            nmm = M if MODE == "full" else M
            for k in range(nmm):
                nk = ns[k]
                nc.tensor.matmul(psum[:, 512 * k:512 * k + nk],
                                 lhs(j1 + k), rhs(j1 + k, nk),
                                 start=True, stop=True,
                                 tile_position=(32 * ((j1 + k) // 128), 0))
        elif MODE in ("dve", "act"):
            n0 = ns[0]
            nc.tensor.matmul(psum[:, 0:n0], lhs(j1), rhs(j1, n0),
                             start=True, stop=True,
                             tile_position=(32 * (j1 // 128), 0))
        if MODE == "pe":
            cur += width
            if cur >= CH:
                flush()
            continue

        path = paths[gi] if MODE == "full" else ("A" if MODE == "dve" else "B")
        if path == "A":
            out_ap = ap2d(stage[:, cur:cur + 1], [[n_pad, M], [1, n_pad]])
            in0_ap = ap2d(psum[:, 0:1], [[512, M], [1, n_pad]])
            in1_ap = ap2d(xse[:, j1 + 1:j1 + 2], [[1, M], [1, n_pad]])
            nc.vector.tensor_mul(out=out_ap, in0=in0_ap, in1=in1_ap)
        else:
            tmp = tmpp.tile([128, 2048], bf16, tag="tmp")
            t_ap = ap2d(tmp[:, 0:1], [[n_pad, M], [1, n_pad]])
            p_ap = ap2d(psum[:, 0:1], [[512, M], [1, n_pad]])
            nc.scalar.copy(out=t_ap, in_=p_ap)
            # even k: x col j1+1+k parity = parity of j1+1; route to the
            # copy where that col sits at an even (4B-aligned) offset.
            for par in range(min(2, M)):
                cnt = (M - par + 1) // 2
                col = j1 + 1 + par
                src, scol = (xse, col) if col % 2 == 0 else (xso, col - 1)
                o_ap = ap2d(stage[:, cur + par * n_pad:cur + par * n_pad + 1],
                            [[2 * n_pad, cnt], [1, n_pad]])
                i0_ap = ap2d(tmp[:, par * n_pad:par * n_pad + 1],
                             [[2 * n_pad, cnt], [1, n_pad]])
                i1_ap = ap2d(src[:, scol:scol + 1], [[2, cnt], [1, n_pad]])
                nc.vector.tensor_mul(out=o_ap, in0=i0_ap, in1=i1_ap)
        cur += width
        if cur >= CH:
            flush()
    flush()


_NC_CACHE = None
_NC_CACHE_KEY = None


def _host_inputs(x, weight):
    i1, i2 = np.triu_indices(NF, k=1)
    wdot = np.einsum("pk,pk->p", weight[i1].astype(np.float64),
                     weight[i2].astype(np.float64)).astype(np.float32)
    wh, wl = _split_bf16(wdot)
    wp_in = {}
    for g in range(4):
        arr = np.zeros((4, _GW[g] + _WPAD), dtype=_bf16)
        sl = slice(_GOFF[g], _GOFF[g + 1])
        arr[0, 0:_GW[g]] = wh[sl]
        arr[1, 0:_GW[g]] = wh[sl]
        arr[2, 0:_GW[g]] = wl[sl]
        arr[3, 0:_GW[g]] = wl[sl]
        wp_in[f"wp{g}"] = arr

    in_maps = []
    for c in range(NCORES):
        xc = x[c * BS:(c + 1) * BS]           # [128, 512] fp32
        xct = np.ascontiguousarray(xc.T)      # [512, 128]
        xh, xl = _split_bf16(xct)
        xt4 = np.empty((4, 4, 128 * 128), dtype=_bf16)
        for g in range(4):
            fh = xh[128 * g:128 * (g + 1)].reshape(-1)
            fl = xl[128 * g:128 * (g + 1)].reshape(-1)
            xt4[g, 0] = fh
            xt4[g, 1] = fl
            xt4[g, 2] = fh
            xt4[g, 3] = fl
        xb = np.zeros((BS, NF + _XPAD), dtype=_bf16)
        xb[:, 0:NF] = xc.astype(_bf16)
        xo = np.zeros((BS, NF + _XPAD), dtype=_bf16)
        xo[:, 0:NF - 1] = xb[:, 1:NF]
        m = {"xt4": xt4, "xse": xb, "xso": xo}
        m.update(wp_in)
        in_maps.append(m)
    return in_maps


def _chunk_widths():
    """Mirror of main_pass flush logic: widths of the DMA'd chunks."""
    widths = []
    cur = 0
    for g in _GROUPS:
        width = g["M"] * g["n_pad"]
        if cur + width > CH:
            widths.append(cur)
            cur = 0
        cur += width
        if cur >= CH:
            widths.append(cur)
            cur = 0
    if cur:
        widths.append(cur)
    return widths


_IDX_CACHE = None


def _pad_index():
    global _IDX_CACHE
    if _IDX_CACHE is None:
        idx = np.empty(P, dtype=np.int64)
        for g in _GROUPS:
            j1, M, n_pad, ns, pp = g["j1"], g["M"], g["n_pad"], g["ns"], g["pp"]
            for k in range(M):
                ot = _off(j1 + k)
                idx[ot:ot + ns[k]] = pp + k * n_pad + np.arange(ns[k])
        _IDX_CACHE = idx
    return _IDX_CACHE


def kernel(x, weight):
    global _NC_CACHE, _NC_CACHE_KEY, LAST_RESULT, _last_in_maps
    x = np.ascontiguousarray(x, dtype=np.float32)
    weight = np.ascontiguousarray(weight, dtype=np.float32)
    assert x.shape == (B, NF) and weight.shape == (NF, K)

    in_maps = _host_inputs(x, weight)
    _last_in_maps = in_maps

    key = (MODE, CH, FLUSH_EVERY, REPS)
    if _NC_CACHE is None or _NC_CACHE_KEY != key:
        _NC_CACHE = _build_nc()
        _NC_CACHE_KEY = key
    nc = _NC_CACHE

    res = bass_utils.run_bass_kernel_spmd(nc, in_maps,
                                          core_ids=list(range(NCORES)),
                                          trace=TRACE)
    LAST_RESULT = {"exec_time_ns": res.exec_time_ns,
                   "trace": res.instructions_and_trace}

    idx = _pad_index()
    out = np.empty((B, P), np.float32)
    for c, r in enumerate(res.results):
        raw = np.asarray(r["out"]).view(np.uint16).reshape(-1)
        # chunk-major contiguous blob -> [128, PPAD] padded rows
        pad = np.empty((BS, PPAD), np.uint16)
        pos = 0
        for w in _chunk_widths():
            pad[:, pos:pos + w] = raw[128 * pos:128 * (pos + w)].reshape(BS, w)
            pos += w
        g = pad[:, idx].astype(np.uint32) << 16
        out[c * BS:(c + 1) * BS] = g.view(np.float32)
    return out
